# revision 26
# baseline (speedup 1.0000x reference)
"""MoEConv Trainium2 kernel (8 NeuronCores, SPMD).

Strategy (dst-sharded, fully dense device program):
- Host: shard destination nodes across 8 cores (degree-balanced), group each
  core's edges by dst node into fixed-window padded "slots" (window uniform
  per 256-slot block, groups never straddle 128-slot halves). Ship per-slot
  x[src] (transposed, bf16) and pos[src]/pos[dst] (f32).
- Device per core, all dense ops:
  * gating: logits = (pos_s - pos_d) @ gate_W + b, top-2 masked softmax ->
    per-slot weight row Kw[slot, 8] (zeros except top-2).
  * Z = x_j @ [W_0|...|W_7]  (one PE matmul per 128-slot tile -> PSUM [128,512])
  * msg = sum_k Kw[:,k] * Z[:,k*64:(k+1)*64]   (DVE mul/add chain)
  * PE pair-transpose msg -> PSUM [128,128]; windowed reduce_max -> ACC cols
  * MLP on ACC (transposed), skip add; host reassembles/unpermutes.
No indirect DMA, no collectives.
"""

import math
from contextlib import ExitStack

import numpy as np

import concourse.bacc as bacc
import concourse.bass as bass
import concourse.tile as tile
from concourse import mybir
from concourse.bass_utils import run_bass_kernel_spmd
from concourse.masks import make_identity

P = 128
N_CORES = 8
IN_C = 64
OUT_C = 64
NK = 8
DIM = 2
BIG = 1.0e30
BF16 = mybir.dt.bfloat16
F32 = mybir.dt.float32


# ---------------------------------------------------------------- host layout
class Layout:
    pass


def build_layout(dst, src, N, T=24):
    """Compute the shared (across cores) block schedule and per-core slot
    arrays. Nodes with degree > T are split into balanced chunks (each chunk
    gets its own acc column; the host max-combines a node's chunk columns).
    Returns Layout with per-core: slot_src, slot_dst (int32 [SL]), and
    shared: block windows w[], caps g[], col offsets, plus output-mapping
    (core, node, half, col) arrays (one entry per chunk)."""
    deg = np.bincount(dst, minlength=N)
    order = np.argsort(-deg, kind="stable")  # global degree-descending
    core_of_node = np.empty(N, dtype=np.int64)
    core_of_node[order] = np.arange(N) % N_CORES

    # per-core node lists (degree-descending)
    nodes_c = [order[core_of_node[order] == c] for c in range(N_CORES)]

    # per-core edge lists grouped by node in list order
    edge_core = core_of_node[dst]
    # rank of node within its core list
    rank_in_core = np.empty(N, dtype=np.int64)
    for c in range(N_CORES):
        rank_in_core[nodes_c[c]] = np.arange(len(nodes_c[c]))

    # group edges: sort each core's edges by rank_in_core[dst]
    edges_c = []
    for c in range(N_CORES):
        idx = np.nonzero(edge_core == c)[0]
        o = np.argsort(rank_in_core[dst[idx]], kind="stable")
        edges_c.append(idx[o])

    # ---- chunkify: split each node into balanced chunks of size <= T ----
    # per core: chunk arrays (node-rank, edge offset within node, size),
    # sorted by chunk size descending (the packing invariant).
    chunks_c = []
    for c in range(N_CORES):
        cn, co, cs = [], [], []
        for r, d in enumerate(deg[nodes_c[c]]):
            k = -(-int(d) // T)
            q, rem = divmod(int(d), k)
            off = 0
            for j in range(k):
                s = q + 1 if j < rem else q
                cn.append(r)
                co.append(off)
                cs.append(s)
                off += s
        cn, co, cs = map(np.asarray, (cn, co, cs))
        o = np.argsort(-cs, kind="stable")
        chunks_c.append((cn[o], co[o], cs[o]))

    # ---- shared block schedule (iterate to fixpoint) ----
    degs_c = [chunks_c[c][2] for c in range(N_CORES)]
    w = []  # shared per-block window

    def pack(core_degs, wseq):
        """Greedily pack nodes into blocks given (possibly partial) wseq.
        Returns list of per-block node counts and the per-block max degree."""
        counts, maxdeg = [], []
        i, nblk = 0, 0
        n = len(core_degs)
        while i < n:
            if nblk < len(wseq):
                wb = max(wseq[nblk], int(core_degs[i]))
            else:
                wb = int(core_degs[i])
            cap = 2 * (P // wb)
            take = min(cap, n - i)
            counts.append(take)
            maxdeg.append(int(core_degs[i]))  # degree-desc => first is max
            i += take
            nblk += 1
        return counts, maxdeg

    for _ in range(20):
        allmax = []
        for c in range(N_CORES):
            _, md = pack(degs_c[c], w)
            allmax.append(md)
        B = max(len(m) for m in allmax)
        neww = []
        for j in range(B):
            cand = [m[j] for m in allmax if j < len(m)]
            wj = max(cand + ([w[j]] if j < len(w) else []))
            neww.append(wj)
        if neww == w:
            break
        w = neww
    w = np.array(w, dtype=np.int64)
    B = len(w)
    g = P // w  # groups (nodes) per 128-half
    colb = np.concatenate([[0], np.cumsum(g)])  # ACC col offset per block
    C = int(colb[-1])

    # ---- per-core slot arrays ----
    lay = Layout()
    lay.B, lay.w, lay.g, lay.colb, lay.C = B, w, g, colb, C
    lay.SL = B * 256
    lay.slot_src, lay.slot_dst = [], []
    lay.out_node, lay.out_half, lay.out_col, lay.out_core = [], [], [], []
    for c in range(N_CORES):
        nodes = nodes_c[c]
        ecs = edges_c[c]
        esrc = src[ecs]
        edst = dst[ecs]
        # edge start offset per node (grouped!)
        dcs = deg[nodes]
        starts = np.concatenate([[0], np.cumsum(dcs)])
        cn, co, cs = chunks_c[c]
        s_src = np.zeros(lay.SL, dtype=np.int32)
        s_dst = np.zeros(lay.SL, dtype=np.int32)
        ni = 0  # chunk cursor
        for b in range(B):
            wb, gb = int(w[b]), int(g[b])
            base = b * 256
            for h in range(2):
                hbase = base + h * P
                for m in range(gb):
                    lo = hbase + m * wb
                    if ni < len(cn):
                        rn = int(cn[ni])
                        st = int(starts[rn] + co[ni])
                        take = int(cs[ni])
                        assert take <= wb
                        s_src[lo:lo + take] = esrc[st:st + take]
                        s_dst[lo:lo + take] = edst[st:st + take]
                        if take < wb:  # pad: duplicate first edge
                            s_src[lo + take:lo + wb] = esrc[st]
                            s_dst[lo + take:lo + wb] = edst[st]
                        lay.out_node.append(nodes[rn])
                        lay.out_half.append(h)
                        lay.out_col.append(colb[b] + m)
                        lay.out_core.append(c)
                        ni += 1
                    else:  # dummy group: duplicate previous slot content
                        s_src[lo:lo + wb] = s_src[lo - 1] if lo > 0 else 0
                        s_dst[lo:lo + wb] = s_dst[lo - 1] if lo > 0 else 0
                # tail pad of the half (128 - gb*wb slots)
                lo = hbase + gb * wb
                if lo < hbase + P:
                    s_src[lo:hbase + P] = s_src[lo - 1] if lo > 0 else 0
                    s_dst[lo:hbase + P] = s_dst[lo - 1] if lo > 0 else 0
        assert ni == len(cn), (ni, len(cn))
        lay.slot_src.append(s_src)
        lay.slot_dst.append(s_dst)
    lay.out_node = np.array(lay.out_node)
    lay.out_half = np.array(lay.out_half)
    lay.out_col = np.array(lay.out_col)
    lay.out_core = np.array(lay.out_core)
    return lay


# ------------------------------------------------------------- device program
GST_DEF = 64  # tiles per gating-supertile DMA (shared with _build_inputs)
KA_DEF = 3   # experts handled by ACT-copy + Pool tensor_tensor scale
RM_DEF = 4   # blocks merged per windowed reduce (same-w runs)


def build_program(lay, repeat=1, skew=3, ka=KA_DEF, rmerge=RM_DEF,
                  GST=GST_DEF, XC=64, zb=3, mb=3):
    T = lay.SL // P          # 128-slot tiles
    B = lay.B                # 256-slot blocks (2 tiles)
    n_gs = math.ceil(T / GST)
    T_pad = n_gs * GST
    BG = GST // 2            # blocks per supertile
    B_pad = n_gs * BG
    C = lay.C
    CP = math.ceil(C / 512) * 512  # padded ACC cols for MLP chunks
    kd = NK - ka

    # uniform-window runs of <= rmerge blocks for merged reduces
    runs = []
    b = 0
    while b < B:
        n = 1
        while n < rmerge and b + n < B and lay.w[b + n] == lay.w[b]:
            n += 1
        runs.append((b, n))
        b += n
    run_of = {}
    for (b0, n) in runs:
        for i in range(n):
            run_of[b0 + i] = (b0, n)

    nc = bacc.Bacc("TRN2", target_bir_lowering=False, debug=False,
                   num_devices=N_CORES)
    # inputs (xjT2: tile pairs stacked on partitions for PE row tiling)
    xjT = nc.dram_tensor("xjT", [2 * IN_C, lay.SL // 2], BF16,
                         kind="ExternalInput")
    kwt = nc.dram_tensor("kwt", [P, T_pad, NK], BF16, kind="ExternalInput")
    wcat = nc.dram_tensor("wcat", [2 * IN_C, NK * OUT_C], BF16,
                          kind="ExternalInput")
    outd = nc.dram_tensor("out", [P, C], F32, kind="ExternalOutput")

    with tile.TileContext(nc) as tc, ExitStack() as ctx:
        cpool = ctx.enter_context(tc.tile_pool(name="consts", bufs=1))
        xpool = ctx.enter_context(tc.tile_pool(name="xc", bufs=2))
        kwpool = ctx.enter_context(tc.tile_pool(name="kw", bufs=3))
        msgp = ctx.enter_context(tc.tile_pool(name="msg", bufs=mb))
        zp = ctx.enter_context(tc.tile_pool(name="z", bufs=zb, space="PSUM"))
        tp = ctx.enter_context(tc.tile_pool(name="tp", bufs=2, space="PSUM"))
        accp = ctx.enter_context(tc.tile_pool(name="acc", bufs=1))

        wcat_s = cpool.tile([2 * IN_C, NK * OUT_C], BF16)
        nc.sync.dma_start(wcat_s[:], wcat[:])
        ident = cpool.tile([P, P], BF16)
        make_identity(nc, ident[:])

        acc = accp.tile([P, C], F32)

        for rep in range(repeat):
         # ---------- gating weights: host-computed, DMA per supertile ----------
         kws = []
         kwas = []
         for gsi in range(n_gs):
            t0 = gsi * GST
            kw16 = kwpool.tile([P, GST, NK], BF16, tag="kw16")
            nc.sync.dma_start(kw16[:], kwt[:, t0:t0 + GST])
            kws.append(kw16)

         # ---------- main loop over blocks (software-pipelined) ----------
         # Emission order per step: Z(b) | zsb/prod(b-1) | tsum(b-2), so the
         # PE queue interleaves Z-matmuls of later blocks ahead of tsum ops
         # that wait on the ACT->DVE chain; tsum(b) consumes a prod finished
         # a full iteration earlier instead of stalling PE on it.
         xc = None
         zs = {}
         prods = {}

         XB = XC // 2  # blocks per x chunk

         def emit_Z(b):
            nonlocal xc
            if b % XB == 0:
                xc = xpool.tile([2 * IN_C, XB * P], BF16)
                lo = b * P
                hi = min(lo + XB * P, lay.SL // 2)
                nc.sync.dma_start(xc[:, :hi - lo], xjT[:, lo:hi])
            z = zp.tile([P, 2, 512], F32, space="PSUM")
            off = (b % XB) * P
            # both tiles of the block run concurrently in disjoint 64-row
            # groups of the PE array (row tiling; contraction is only 64)
            for i in range(2):
                nc.tensor.matmul(
                    out=z[:, i],
                    lhsT=xc[i * IN_C:(i + 1) * IN_C, off:off + P],
                    rhs=wcat_s[i * IN_C:(i + 1) * IN_C, :],
                    start=True, stop=True)
            zs[b] = z

         def emit_prod(b):
            t0 = 2 * b
            z = zs.pop(b)
            kw16 = kws[t0 // GST]
            gg = t0 % GST  # first tile's group index within supertile
            sp = ka * OUT_C
            # ACT: stage experts [0, ka) to SBUF in k-major layout
            zsba = msgp.tile([P, ka, 2, OUT_C], BF16, tag="zsba")
            nc.scalar.copy(out=zsba[:].rearrange("p k a c -> p a k c"),
                           in_=z[:, :, :sp])
            # Pool: gating multiply on the staged experts (broadcast in1)
            proda = msgp.tile([P, ka, 2, OUT_C], BF16, tag="proda")
            kwba = kw16[:, gg:gg + 2, :ka].rearrange(
                "p a k -> p k a")[:, :, :, None].to_broadcast(
                [P, ka, 2, OUT_C])
            nc.gpsimd.tensor_tensor(out=proda[:], in0=zsba[:], in1=kwba,
                                    op=mybir.AluOpType.mult)
            # DVE: scaled readout of experts [ka, 8) straight from PSUM
            prodd = msgp.tile([P, kd, 2, OUT_C], BF16, tag="prodd")
            kwb = kw16[:, gg:gg + 2, ka:].rearrange(
                "p a k -> p k a")[:, :, :, None].to_broadcast(
                [P, kd, 2, OUT_C])
            nc.vector.tensor_tensor(
                out=prodd[:],
                in0=z[:, :, sp:].rearrange("p a (k c) -> p k a c", k=kd),
                in1=kwb, op=mybir.AluOpType.mult)
            prods[b] = (proda, prodd)

         tps_runs = {}

         def emit_tsum(b):
            # sum over k via accumulating transpose-matmuls -> msg^T pair
            proda, prodd = prods.pop(b)
            b0, rn = run_of[b]
            if b == b0:
                tps_runs[b0] = tp.tile([P, rmerge, P], F32, space="PSUM",
                                       tag="tps", name=f"tpsr{b0}")
            tps = tps_runs[b0]
            ri = b - b0
            for k in range(NK):
                lhsT = proda[:, k] if k < ka else prodd[:, k - ka]
                nc.tensor.matmul(
                    out=tps[:, ri], lhsT=lhsT.rearrange("p a c -> p (a c)"),
                    rhs=ident[:], start=(k == 0), stop=(k == NK - 1))
            if ri == rn - 1:
                wb, gb = int(lay.w[b0]), int(lay.g[b0])
                cb = int(lay.colb[b0])
                tpsr = tps_runs.pop(b0)
                nc.vector.tensor_reduce(
                    out=acc[:, cb:cb + rn * gb].rearrange(
                        "p (r g) -> p r g", r=rn),
                    in_=tpsr[:, :rn, :gb * wb].rearrange(
                        "p r (g w) -> p r g w", g=gb),
                    axis=mybir.AxisListType.X, op=mybir.AluOpType.max)

         for step in range(B + skew):
            if step < B:
                emit_Z(step)
            if skew == 0:
                emit_prod(step)
                emit_tsum(step)
            else:
                if 0 <= step - 1 < B:
                    emit_prod(step - 1)
                if 0 <= step - skew:
                    emit_tsum(step - skew)

        # ---------- writeback (MLP + chunk combine happen on host) ----------
        nc.sync.dma_start(outd[:, :], acc[:, :])

    nc.compile()
    return nc


# ------------------------------------------------------------------ top level
def _build_inputs(lay, x, pos, expert_weights, gate_W, gate_b, W1, W2,
                  ka=KA_DEF):
    T = lay.SL // P
    GST = GST_DEF
    n_gs = math.ceil(T / GST)
    T_pad = n_gs * GST
    B = T // 2
    B_pad = n_gs * (GST // 2)
    wcat = np.ascontiguousarray(
        expert_weights.transpose(1, 0, 2).reshape(IN_C, NK * OUT_C)
    ).astype(np.float32)
    import ml_dtypes
    in_maps = []
    for c in range(N_CORES):
        ssrc = lay.slot_src[c]
        sdst = lay.slot_dst[c]
        xjT = np.ascontiguousarray(x[ssrc].T).astype(ml_dtypes.bfloat16)
        # stack tile pairs on partitions: [128, SL/2], row a*64+c
        xjT = np.ascontiguousarray(
            xjT.reshape(IN_C, B, 2, P).transpose(2, 0, 1, 3).reshape(
                2 * IN_C, lay.SL // 2))
        # host gating: per-slot sparse weight row kw[slot, 8] (top-2 softmax)
        logits = (pos[ssrc] - pos[sdst]) @ gate_W.astype(np.float64) + gate_b
        order = np.argsort(-logits, axis=1, kind="stable")
        k1, k2 = order[:, 0], order[:, 1]
        v1 = np.take_along_axis(logits, k1[:, None], 1)[:, 0]
        v2 = np.take_along_axis(logits, k2[:, None], 1)[:, 0]
        e = np.exp(v2 - v1)
        w1g = 1.0 / (1.0 + e)
        kw = np.zeros(logits.shape, np.float32)
        np.put_along_axis(kw, k1[:, None], w1g[:, None].astype(np.float32), 1)
        np.put_along_axis(kw, k2[:, None],
                          (e * w1g)[:, None].astype(np.float32), 1)
        kwt = kw.reshape(T, P, NK).transpose(1, 0, 2)
        if T_pad > T:
            kwt = np.concatenate(
                [kwt, np.zeros((P, T_pad - T, NK), kwt.dtype)], axis=1)
        kwt = np.ascontiguousarray(kwt).astype(ml_dtypes.bfloat16)
        wcat2 = np.concatenate([wcat, wcat], axis=0)
        in_maps.append({
            "xjT": xjT,
            "kwt": kwt,
            "wcat": wcat2.astype(ml_dtypes.bfloat16),
        })
    return in_maps


def kernel(x, pos, edge_index, expert_weights, gate_W, gate_b, W1, W2):
    x = np.asarray(x, dtype=np.float32)
    pos = np.asarray(pos, dtype=np.float32)
    ei = np.asarray(edge_index)
    N = x.shape[0]
    dst = ei[:, 0].astype(np.int64)
    src = ei[:, 1].astype(np.int64)

    lay = build_layout(dst, src, N)
    nc = build_program(lay)
    in_maps = _build_inputs(lay, x, pos, np.asarray(expert_weights),
                            np.asarray(gate_W), np.asarray(gate_b),
                            np.asarray(W1), np.asarray(W2))
    res = run_bass_kernel_spmd(nc, in_maps, list(range(N_CORES)))
    return finish(lay, [res.results[c]["out"] for c in range(N_CORES)], N,
                  np.asarray(W1), np.asarray(W2))


def finish(lay, outs, N, W1, W2):
    """Host-side tail: max-combine each node's chunk columns, then the
    global MLP with skip (must act on the combined max, so it lives here)."""
    out = np.full((N, OUT_C), -np.inf, dtype=np.float32)
    for c in range(N_CORES):
        o = np.asarray(outs[c])  # [128, C]
        sel = lay.out_core == c
        nodes = lay.out_node[sel]
        halves = lay.out_half[sel]
        cols = lay.out_col[sel]
        for h in range(2):
            m = halves == h
            np.maximum.at(out, nodes[m], o[h * OUT_C:(h + 1) * OUT_C,
                                           cols[m]].T)
    out[np.isneginf(out)] = 0.0
    h = np.maximum(out @ W1.astype(np.float32), 0.0) @ W2.astype(np.float32)
    return (h + out).astype(np.float32)



# revision 27
# speedup vs baseline: 2.6236x; 2.6236x over previous
"""MoEConv Trainium2 kernel (8 NeuronCores, SPMD).

Strategy (dst-sharded, fully dense device program):
- Host: shard destination nodes across 8 cores (degree-balanced), group each
  core's edges by dst node into fixed-window padded "slots" (window uniform
  per 256-slot block, groups never straddle 128-slot halves). Ship per-slot
  x[src] (transposed, bf16) and pos[src]/pos[dst] (f32).
- Device per core, all dense ops:
  * gating: logits = (pos_s - pos_d) @ gate_W + b, top-2 masked softmax ->
    per-slot weight row Kw[slot, 8] (zeros except top-2).
  * Z = x_j @ [W_0|...|W_7]  (one PE matmul per 128-slot tile -> PSUM [128,512])
  * msg = sum_k Kw[:,k] * Z[:,k*64:(k+1)*64]   (DVE mul/add chain)
  * PE pair-transpose msg -> PSUM [128,128]; windowed reduce_max -> ACC cols
  * MLP on ACC (transposed), skip add; host reassembles/unpermutes.
No indirect DMA, no collectives.
"""

import math
from contextlib import ExitStack

import numpy as np

import concourse.bacc as bacc
import concourse.bass as bass
import concourse.tile as tile
from concourse import mybir
from concourse.bass_utils import run_bass_kernel_spmd
from concourse.masks import make_identity

P = 128
N_CORES = 8
IN_C = 64
OUT_C = 64
NK = 8
DIM = 2
BIG = 1.0e30
BF16 = mybir.dt.bfloat16
F32 = mybir.dt.float32


# ---------------------------------------------------------------- host layout
class Layout:
    pass


def build_layout(dst, src, N, T=24):
    """Compute the shared (across cores) block schedule and per-core slot
    arrays. Nodes with degree > T are split into balanced chunks (each chunk
    gets its own acc column; the host max-combines a node's chunk columns).
    Returns Layout with per-core: slot_src, slot_dst (int32 [SL]), and
    shared: block windows w[], caps g[], col offsets, plus output-mapping
    (core, node, half, col) arrays (one entry per chunk)."""
    deg = np.bincount(dst, minlength=N)
    order = np.argsort(-deg, kind="stable")  # global degree-descending
    core_of_node = np.empty(N, dtype=np.int64)
    core_of_node[order] = np.arange(N) % N_CORES

    # per-core node lists (degree-descending)
    nodes_c = [order[core_of_node[order] == c] for c in range(N_CORES)]

    # per-core edge lists grouped by node in list order
    edge_core = core_of_node[dst]
    # rank of node within its core list
    rank_in_core = np.empty(N, dtype=np.int64)
    for c in range(N_CORES):
        rank_in_core[nodes_c[c]] = np.arange(len(nodes_c[c]))

    # group edges: sort each core's edges by rank_in_core[dst]
    edges_c = []
    for c in range(N_CORES):
        idx = np.nonzero(edge_core == c)[0]
        o = np.argsort(rank_in_core[dst[idx]], kind="stable")
        edges_c.append(idx[o])

    # ---- chunkify: split each node into balanced chunks of size <= T ----
    # per core: chunk arrays (node-rank, edge offset within node, size),
    # sorted by chunk size descending (the packing invariant).
    chunks_c = []
    for c in range(N_CORES):
        cn, co, cs = [], [], []
        for r, d in enumerate(deg[nodes_c[c]]):
            k = -(-int(d) // T)
            q, rem = divmod(int(d), k)
            off = 0
            for j in range(k):
                s = q + 1 if j < rem else q
                cn.append(r)
                co.append(off)
                cs.append(s)
                off += s
        cn, co, cs = map(np.asarray, (cn, co, cs))
        o = np.argsort(-cs, kind="stable")
        chunks_c.append((cn[o], co[o], cs[o]))

    # ---- shared block schedule (iterate to fixpoint) ----
    degs_c = [chunks_c[c][2] for c in range(N_CORES)]
    w = []  # shared per-block window

    def pack(core_degs, wseq):
        """Greedily pack nodes into blocks given (possibly partial) wseq.
        Returns list of per-block node counts and the per-block max degree."""
        counts, maxdeg = [], []
        i, nblk = 0, 0
        n = len(core_degs)
        while i < n:
            if nblk < len(wseq):
                wb = max(wseq[nblk], int(core_degs[i]))
            else:
                wb = int(core_degs[i])
            cap = 2 * (P // wb)
            take = min(cap, n - i)
            counts.append(take)
            maxdeg.append(int(core_degs[i]))  # degree-desc => first is max
            i += take
            nblk += 1
        return counts, maxdeg

    for _ in range(20):
        allmax = []
        for c in range(N_CORES):
            _, md = pack(degs_c[c], w)
            allmax.append(md)
        B = max(len(m) for m in allmax)
        neww = []
        for j in range(B):
            cand = [m[j] for m in allmax if j < len(m)]
            wj = max(cand + ([w[j]] if j < len(w) else []))
            neww.append(wj)
        if neww == w:
            break
        w = neww
    w = np.array(w, dtype=np.int64)
    B = len(w)
    g = P // w  # groups (nodes) per 128-half
    colb = np.concatenate([[0], np.cumsum(g)])  # ACC col offset per block
    C = int(colb[-1])

    # ---- per-core slot arrays ----
    lay = Layout()
    lay.B, lay.w, lay.g, lay.colb, lay.C = B, w, g, colb, C
    lay.SL = B * 256
    lay.slot_src, lay.slot_dst = [], []
    lay.out_node, lay.out_half, lay.out_col, lay.out_core = [], [], [], []
    for c in range(N_CORES):
        nodes = nodes_c[c]
        ecs = edges_c[c]
        esrc = src[ecs]
        edst = dst[ecs]
        # edge start offset per node (grouped!)
        dcs = deg[nodes]
        starts = np.concatenate([[0], np.cumsum(dcs)])
        cn, co, cs = chunks_c[c]
        s_src = np.zeros(lay.SL, dtype=np.int32)
        s_dst = np.zeros(lay.SL, dtype=np.int32)
        ni = 0  # chunk cursor
        for b in range(B):
            wb, gb = int(w[b]), int(g[b])
            base = b * 256
            for h in range(2):
                hbase = base + h * P
                for m in range(gb):
                    lo = hbase + m * wb
                    if ni < len(cn):
                        rn = int(cn[ni])
                        st = int(starts[rn] + co[ni])
                        take = int(cs[ni])
                        assert take <= wb
                        s_src[lo:lo + take] = esrc[st:st + take]
                        s_dst[lo:lo + take] = edst[st:st + take]
                        if take < wb:  # pad: duplicate first edge
                            s_src[lo + take:lo + wb] = esrc[st]
                            s_dst[lo + take:lo + wb] = edst[st]
                        lay.out_node.append(nodes[rn])
                        lay.out_half.append(h)
                        lay.out_col.append(colb[b] + m)
                        lay.out_core.append(c)
                        ni += 1
                    else:  # dummy group: duplicate previous slot content
                        s_src[lo:lo + wb] = s_src[lo - 1] if lo > 0 else 0
                        s_dst[lo:lo + wb] = s_dst[lo - 1] if lo > 0 else 0
                # tail pad of the half (128 - gb*wb slots)
                lo = hbase + gb * wb
                if lo < hbase + P:
                    s_src[lo:hbase + P] = s_src[lo - 1] if lo > 0 else 0
                    s_dst[lo:hbase + P] = s_dst[lo - 1] if lo > 0 else 0
        assert ni == len(cn), (ni, len(cn))
        lay.slot_src.append(s_src)
        lay.slot_dst.append(s_dst)
    lay.out_node = np.array(lay.out_node)
    lay.out_half = np.array(lay.out_half)
    lay.out_col = np.array(lay.out_col)
    lay.out_core = np.array(lay.out_core)
    return lay


# ------------------------------------------------------------- device program
GST_DEF = 64  # tiles per gating-supertile DMA (shared with _build_inputs)
KA_DEF = 3   # experts handled by ACT-copy + Pool tensor_tensor scale
RM_DEF = 4   # blocks merged per windowed reduce (same-w runs)


def build_program(lay, repeat=1, skew=3, ka=KA_DEF, rmerge=RM_DEF,
                  GST=GST_DEF, XC=64, zb=3, mb=3):
    T = lay.SL // P          # 128-slot tiles
    B = lay.B                # 256-slot blocks (2 tiles)
    n_gs = math.ceil(T / GST)
    T_pad = n_gs * GST
    BG = GST // 2            # blocks per supertile
    B_pad = n_gs * BG
    C = lay.C
    CP = math.ceil(C / 512) * 512  # padded ACC cols for MLP chunks
    kd = NK - ka

    # uniform-window runs of <= rmerge blocks for merged reduces
    runs = []
    b = 0
    while b < B:
        n = 1
        while n < rmerge and b + n < B and lay.w[b + n] == lay.w[b]:
            n += 1
        runs.append((b, n))
        b += n
    run_of = {}
    for (b0, n) in runs:
        for i in range(n):
            run_of[b0 + i] = (b0, n)

    nc = bacc.Bacc("TRN2", target_bir_lowering=False, debug=False,
                   num_devices=N_CORES)
    # inputs (xjT2: tile pairs stacked on partitions for PE row tiling)
    xjT = nc.dram_tensor("xjT", [2 * IN_C, lay.SL // 2], BF16,
                         kind="ExternalInput")
    kwt = nc.dram_tensor("kwt", [P, T_pad, NK], BF16, kind="ExternalInput")
    wcat = nc.dram_tensor("wcat", [2 * IN_C, NK * OUT_C], BF16,
                          kind="ExternalInput")
    outd = nc.dram_tensor("out", [P, C], F32, kind="ExternalOutput")

    with tile.TileContext(nc) as tc, ExitStack() as ctx:
        cpool = ctx.enter_context(tc.tile_pool(name="consts", bufs=1))
        xpool = ctx.enter_context(tc.tile_pool(name="xc", bufs=2))
        kwpool = ctx.enter_context(tc.tile_pool(name="kw", bufs=3))
        msgp = ctx.enter_context(tc.tile_pool(name="msg", bufs=mb))
        zp = ctx.enter_context(tc.tile_pool(name="z", bufs=zb, space="PSUM"))
        tp = ctx.enter_context(tc.tile_pool(name="tp", bufs=2, space="PSUM"))
        accp = ctx.enter_context(tc.tile_pool(name="acc", bufs=1))

        wcat_s = cpool.tile([2 * IN_C, NK * OUT_C], BF16)
        nc.sync.dma_start(wcat_s[:], wcat[:])
        ident = cpool.tile([P, P], BF16)
        make_identity(nc, ident[:])

        acc = accp.tile([P, C], F32)

        for rep in range(repeat):
         # ---------- gating weights: host-computed, DMA per supertile ----------
         kws = []
         kwas = []
         for gsi in range(n_gs):
            t0 = gsi * GST
            kw16 = kwpool.tile([P, GST, NK], BF16, tag="kw16")
            nc.sync.dma_start(kw16[:], kwt[:, t0:t0 + GST])
            kws.append(kw16)

         # ---------- main loop over blocks (software-pipelined) ----------
         # Emission order per step: Z(b) | zsb/prod(b-1) | tsum(b-2), so the
         # PE queue interleaves Z-matmuls of later blocks ahead of tsum ops
         # that wait on the ACT->DVE chain; tsum(b) consumes a prod finished
         # a full iteration earlier instead of stalling PE on it.
         xc = None
         zs = {}
         prods = {}

         XB = XC // 2  # blocks per x chunk

         def emit_Z(b):
            nonlocal xc
            if b % XB == 0:
                xc = xpool.tile([2 * IN_C, XB * P], BF16)
                lo = b * P
                hi = min(lo + XB * P, lay.SL // 2)
                nc.sync.dma_start(xc[:, :hi - lo], xjT[:, lo:hi])
            z = zp.tile([P, 2, 512], F32, space="PSUM")
            off = (b % XB) * P
            # both tiles of the block run concurrently in disjoint 64-row
            # groups of the PE array (row tiling; contraction is only 64)
            for i in range(2):
                nc.tensor.matmul(
                    out=z[:, i],
                    lhsT=xc[i * IN_C:(i + 1) * IN_C, off:off + P],
                    rhs=wcat_s[i * IN_C:(i + 1) * IN_C, :],
                    start=True, stop=True)
            zs[b] = z

         def emit_prod(b):
            t0 = 2 * b
            z = zs.pop(b)
            kw16 = kws[t0 // GST]
            gg = t0 % GST  # first tile's group index within supertile
            sp = ka * OUT_C
            # ACT: stage experts [0, ka) to SBUF in k-major layout
            zsba = msgp.tile([P, ka, 2, OUT_C], BF16, tag="zsba")
            nc.scalar.copy(out=zsba[:].rearrange("p k a c -> p a k c"),
                           in_=z[:, :, :sp])
            # Pool: gating multiply on the staged experts (broadcast in1)
            proda = msgp.tile([P, ka, 2, OUT_C], BF16, tag="proda")
            kwba = kw16[:, gg:gg + 2, :ka].rearrange(
                "p a k -> p k a")[:, :, :, None].to_broadcast(
                [P, ka, 2, OUT_C])
            nc.gpsimd.tensor_tensor(out=proda[:], in0=zsba[:], in1=kwba,
                                    op=mybir.AluOpType.mult)
            # DVE: scaled readout of experts [ka, 8) straight from PSUM
            prodd = msgp.tile([P, kd, 2, OUT_C], BF16, tag="prodd")
            kwb = kw16[:, gg:gg + 2, ka:].rearrange(
                "p a k -> p k a")[:, :, :, None].to_broadcast(
                [P, kd, 2, OUT_C])
            nc.vector.tensor_tensor(
                out=prodd[:],
                in0=z[:, :, sp:].rearrange("p a (k c) -> p k a c", k=kd),
                in1=kwb, op=mybir.AluOpType.mult)
            prods[b] = (proda, prodd)

         tps_runs = {}

         def emit_tsum(b):
            # sum over k via accumulating transpose-matmuls -> msg^T pair
            proda, prodd = prods.pop(b)
            b0, rn = run_of[b]
            if b == b0:
                tps_runs[b0] = tp.tile([P, rmerge, P], F32, space="PSUM",
                                       tag="tps", name=f"tpsr{b0}")
            tps = tps_runs[b0]
            ri = b - b0
            for k in range(NK):
                lhsT = proda[:, k] if k < ka else prodd[:, k - ka]
                nc.tensor.matmul(
                    out=tps[:, ri], lhsT=lhsT.rearrange("p a c -> p (a c)"),
                    rhs=ident[:], start=(k == 0), stop=(k == NK - 1))
            if ri == rn - 1:
                wb, gb = int(lay.w[b0]), int(lay.g[b0])
                cb = int(lay.colb[b0])
                tpsr = tps_runs.pop(b0)
                nc.vector.tensor_reduce(
                    out=acc[:, cb:cb + rn * gb].rearrange(
                        "p (r g) -> p r g", r=rn),
                    in_=tpsr[:, :rn, :gb * wb].rearrange(
                        "p r (g w) -> p r g w", g=gb),
                    axis=mybir.AxisListType.X, op=mybir.AluOpType.max)

         for step in range(B + skew):
            if step < B:
                emit_Z(step)
            if skew == 0:
                emit_prod(step)
                emit_tsum(step)
            else:
                if 0 <= step - 1 < B:
                    emit_prod(step - 1)
                if 0 <= step - skew:
                    emit_tsum(step - skew)

        # ---------- writeback (MLP + chunk combine happen on host) ----------
        # chunked so early column ranges stream out while later blocks compute
        NW = 8
        for i in range(NW):
            lo = (C * i) // NW
            hi = (C * (i + 1)) // NW
            nc.sync.dma_start(outd[:, lo:hi], acc[:, lo:hi])

    nc.compile()
    return nc


# ------------------------------------------------------------------ top level
def _build_inputs(lay, x, pos, expert_weights, gate_W, gate_b, W1, W2,
                  ka=KA_DEF):
    T = lay.SL // P
    GST = GST_DEF
    n_gs = math.ceil(T / GST)
    T_pad = n_gs * GST
    B = T // 2
    B_pad = n_gs * (GST // 2)
    wcat = np.ascontiguousarray(
        expert_weights.transpose(1, 0, 2).reshape(IN_C, NK * OUT_C)
    ).astype(np.float32)
    import ml_dtypes
    in_maps = []
    for c in range(N_CORES):
        ssrc = lay.slot_src[c]
        sdst = lay.slot_dst[c]
        xjT = np.ascontiguousarray(x[ssrc].T).astype(ml_dtypes.bfloat16)
        # stack tile pairs on partitions: [128, SL/2], row a*64+c
        xjT = np.ascontiguousarray(
            xjT.reshape(IN_C, B, 2, P).transpose(2, 0, 1, 3).reshape(
                2 * IN_C, lay.SL // 2))
        # host gating: per-slot sparse weight row kw[slot, 8] (top-2 softmax)
        logits = (pos[ssrc] - pos[sdst]) @ gate_W.astype(np.float64) + gate_b
        order = np.argsort(-logits, axis=1, kind="stable")
        k1, k2 = order[:, 0], order[:, 1]
        v1 = np.take_along_axis(logits, k1[:, None], 1)[:, 0]
        v2 = np.take_along_axis(logits, k2[:, None], 1)[:, 0]
        e = np.exp(v2 - v1)
        w1g = 1.0 / (1.0 + e)
        kw = np.zeros(logits.shape, np.float32)
        np.put_along_axis(kw, k1[:, None], w1g[:, None].astype(np.float32), 1)
        np.put_along_axis(kw, k2[:, None],
                          (e * w1g)[:, None].astype(np.float32), 1)
        kwt = kw.reshape(T, P, NK).transpose(1, 0, 2)
        if T_pad > T:
            kwt = np.concatenate(
                [kwt, np.zeros((P, T_pad - T, NK), kwt.dtype)], axis=1)
        kwt = np.ascontiguousarray(kwt).astype(ml_dtypes.bfloat16)
        wcat2 = np.concatenate([wcat, wcat], axis=0)
        in_maps.append({
            "xjT": xjT,
            "kwt": kwt,
            "wcat": wcat2.astype(ml_dtypes.bfloat16),
        })
    return in_maps


def kernel(x, pos, edge_index, expert_weights, gate_W, gate_b, W1, W2):
    x = np.asarray(x, dtype=np.float32)
    pos = np.asarray(pos, dtype=np.float32)
    ei = np.asarray(edge_index)
    N = x.shape[0]
    dst = ei[:, 0].astype(np.int64)
    src = ei[:, 1].astype(np.int64)

    lay = build_layout(dst, src, N)
    nc = build_program(lay)
    in_maps = _build_inputs(lay, x, pos, np.asarray(expert_weights),
                            np.asarray(gate_W), np.asarray(gate_b),
                            np.asarray(W1), np.asarray(W2))
    res = run_bass_kernel_spmd(nc, in_maps, list(range(N_CORES)))
    return finish(lay, [res.results[c]["out"] for c in range(N_CORES)], N,
                  np.asarray(W1), np.asarray(W2))


def finish(lay, outs, N, W1, W2):
    """Host-side tail: max-combine each node's chunk columns, then the
    global MLP with skip (must act on the combined max, so it lives here)."""
    out = np.full((N, OUT_C), -np.inf, dtype=np.float32)
    for c in range(N_CORES):
        o = np.asarray(outs[c])  # [128, C]
        sel = lay.out_core == c
        nodes = lay.out_node[sel]
        halves = lay.out_half[sel]
        cols = lay.out_col[sel]
        for h in range(2):
            m = halves == h
            np.maximum.at(out, nodes[m], o[h * OUT_C:(h + 1) * OUT_C,
                                           cols[m]].T)
    out[np.isneginf(out)] = 0.0
    h = np.maximum(out @ W1.astype(np.float32), 0.0) @ W2.astype(np.float32)
    return (h + out).astype(np.float32)

